# revision 27
# baseline (speedup 1.0000x reference)
"""Trainium2 Bass kernel for nn_BuildModel_3796751089773.

RAIM-attention + LSTMCell scan over T=256 steps, B=1024, F=128, H=256, W=3,
followed by sum-over-time prediction head -> [B, 1].

Strategy (8 cores, data-parallel over batch, B_local = 128 = SBUF partitions):
  - Normal layout [batch_partitions, feature_free] for attention softmax and
    all elementwise work (per-partition scalars make softmax/z cheap).
  - gates = z @ W_ih^T + h @ W_hh^T computed with activations-transposed as
    PE stationary (hT/zT via PE transposes), weights streaming as rhs.
  - Output head sum_t(h_t) @ w_pred^T accumulated in a persistent PSUM bank
    by riding tiny N=1 matmuls on the already-loaded hT stationaries.
  - sigmoid(x) = 0.5*(1+tanh(x/2)) so the only ACT functions used are
    tanh/exp/copy -> one ACT table set ("exp_and_others"), loaded once.
  - h,c state kept doubled (H=2h, C=2c) so the LSTM update is exactly three
    fused scalar_tensor_tensor ops; the 0.5 factors fold into weights.
  - Critical-path schedule (the scan is latency-bound, all engines <55%):
    * x-dependent parts of alpha/beta preacts for step t+1 are fully
      accumulated in step t's tail (3 static rhs matmuls vs transposed x),
      so at step t's head only the two h@wab matmuls stand before the
      softmax chain -- the second one carries the PSUM stop.
    * alpha path (3 cols) is computed before the beta path (128 cols) so
      the vector u-chain starts ~200ns earlier; S_a comes from two 1-col
      vector adds instead of a serializing accumulator read.
    * gates PSUM is split g_lo=[f,i] / g_hi=[o,g] so tanh(f,i) fires on
      the first z-matmul stop, not the last.
    * h transposes + hT copies (scalar||vector in parallel) + next-step
      wab/gates-h/y matmuls all live in the tail, overlapping the next
      step's softmax chain on the PE.
  - fp32 storage everywhere; matmuls run as float32r (full-rate for N>=256).
"""

import os
import sys

import numpy as np

for _p in ("/opt/trn_rl_repo",):
    if _p not in sys.path:
        sys.path.insert(0, _p)

import concourse.bass as bass
import concourse.bacc as bacc
import concourse.tile as tile
from concourse import mybir
from concourse.bass_utils import run_bass_kernel_spmd
from concourse.masks import make_identity
from concourse.dve_ops import (
    OPS as _DVE_OPS, CUSTOM_DVE_SPECS as _DVE_SPECS,
    _SUB_OPCODE_FOR_NAME as _DVE_ROWS, _CUSTOM_DVE_ROW_BASE as _DVE_ROW_BASE,
    DveOp as _DveOp,
)
from concourse.dve_spec import Spec as _Spec, Src0 as _Src0, Src1 as _Src1, \
    C0 as _C0, C1 as _C1, lower as _dve_lower
from concourse.dve_uop import DveOpSpec as _DveOpSpec


def _register_u2_op():
    """out = in0*s0 + in1*s1 with two per-partition scalars (one DVE inst)."""
    name = "U2_MULADD_ANT"
    if name in _DVE_ROWS:
        return next(o for o in _DVE_OPS if o.name == name)
    spec = _Spec(
        body=_Src0 * _C0 + _Src1 * _C1,
        reference=lambda in0, in1, s0, s1, imm2:
            in0.astype(np.float32) * s0 + in1.astype(np.float32) * s1,
    )
    row = _DVE_ROW_BASE + len(_DVE_OPS)
    _DVE_ROWS[name] = row
    shas = {}
    for ver in ("v3", "v4"):
        try:
            uops = _dve_lower(spec, ver=ver)
            shas[ver] = _DveOpSpec(name=name, opcode=row, uops=uops,
                                   rd1_en=True).sha(ver)
        except Exception:
            pass
    op = _DveOp(name, spec, subdim=False, uops_sha=shas)
    _DVE_OPS.append(op)
    _DVE_SPECS[name] = spec
    return op


U2_OP = _register_u2_op()

B, T, F, W, H, L = 1024, 256, 128, 3, 256, 1
NCORES = 8
BL = B // NCORES  # 128
AF = mybir.ActivationFunctionType
ALU = mybir.AluOpType
DT = mybir.dt
F32 = DT.float32
F32R = DT.float32r

_CACHE = {}
FILL_Z = int(os.environ.get("BASS_FILL_Z", "6"))
FILL_H = int(os.environ.get("BASS_FILL_H", "3"))
FILL_E = int(os.environ.get("BASS_FILL_E", "2"))


def build_kernel(gate_bias_nonzero, ab_bias_nonzero):
    nc = bacc.Bacc("TRN2")

    xn_d = nc.dram_tensor("xn", [T, BL, F], F32, kind="ExternalInput")
    xt_d = nc.dram_tensor("xt", [T, F, BL], F32R, kind="ExternalInput")
    wg_d = nc.dram_tensor("wg", [3, 128, 1024], F32R, kind="ExternalInput")
    wab_d = nc.dram_tensor("wab", [2, 128, 256], F32R, kind="ExternalInput")
    wr_d = nc.dram_tensor("wr", [3, 128, 256], F32R, kind="ExternalInput")
    wp_d = nc.dram_tensor("wp", [2, 128, 8], F32R, kind="ExternalInput")
    bg_d = nc.dram_tensor("bg", [1, 1024], F32R, kind="ExternalInput")
    bab_d = nc.dram_tensor("bab", [1, 256], F32R, kind="ExternalInput")
    y_d = nc.dram_tensor("y", [BL, L], F32, kind="ExternalOutput")

    from contextlib import ExitStack

    GB = 4  # steps per DMA group
    NGROUPS = T // GB

    with tile.TileContext(nc) as tc, ExitStack() as ctx:
        singles = ctx.enter_context(tc.tile_pool(name="singles", bufs=1))
        xn_pool = ctx.enter_context(tc.tile_pool(name="xn", bufs=3))
        xt_pool = ctx.enter_context(tc.tile_pool(name="xt", bufs=3))
        work = ctx.enter_context(tc.tile_pool(name="work", bufs=2))
        ab_pool = ctx.enter_context(tc.tile_pool(name="abps", bufs=2, space="PSUM"))
        g_pool = ctx.enter_context(tc.tile_pool(name="gps", bufs=1, space="PSUM"))
        tr_pool = ctx.enter_context(tc.tile_pool(name="trps", bufs=1, space="PSUM"))
        y_pool = ctx.enter_context(tc.tile_pool(name="yps", bufs=1, space="PSUM"))
        fill_pool = ctx.enter_context(tc.tile_pool(name="fill", bufs=1, space="PSUM"))

        # ---- one-time loads -------------------------------------------------
        wg_s = singles.tile([128, 3, 1024], F32R)
        wab_s = singles.tile([128, 2, 256], F32R)
        wr_s = singles.tile([128, 3, 256], F32R)
        wp_s = singles.tile([128, 2, 8], F32R)
        for k in range(3):
            nc.sync.dma_start(out=wg_s[:, k, :], in_=wg_d[k])
            nc.sync.dma_start(out=wr_s[:, k, :], in_=wr_d[k])
        for k in range(2):
            nc.sync.dma_start(out=wab_s[:, k, :], in_=wab_d[k])
            nc.sync.dma_start(out=wp_s[:, k, :], in_=wp_d[k])
        ident = singles.tile([128, 128], F32)
        make_identity(nc, ident)
        ones_row = None
        bg_s = bab_s = None
        if gate_bias_nonzero or ab_bias_nonzero:
            ones_row = singles.tile([1, 128], F32R)
            nc.vector.memset(ones_row, 1.0)
        if gate_bias_nonzero:
            bg_s = singles.tile([1, 1024], F32R)
            nc.sync.dma_start(out=bg_s, in_=bg_d[:])
        if ab_bias_nonzero:
            bab_s = singles.tile([1, 256], F32R)
            nc.sync.dma_start(out=bab_s, in_=bab_d[:])

        # ---- persistent state ----------------------------------------------
        C_s = singles.tile([128, 256], F32)  # doubled cell state 2*c
        nc.gpsimd.memset(C_s, 0.0)

        y_ps = y_pool.tile([128, 8], F32)

        xn_groups = {}
        xt_groups = {}

        def dma_group(gi):
            t0 = gi * GB
            gxn = xn_pool.tile([128, GB, 128], F32, tag="xn", name=f"xng{gi}")
            nc.sync.dma_start(out=gxn,
                              in_=xn_d[t0:t0 + GB].rearrange("k p f -> p k f"))
            xn_groups[gi] = gxn
            gxt = xt_pool.tile([128, GB, 128], F32R, tag="xt", name=f"xtg{gi}")
            nc.sync.dma_start(out=gxt,
                              in_=xt_d[t0:t0 + GB].rearrange("k p f -> p k f"))
            xt_groups[gi] = gxt

        def xn_t(t):
            return xn_groups[t // GB][:, t % GB, :]

        def xt_t(t):
            return xt_groups[t // GB][:, t % GB, :]

        dma_group(0)

        ab_tiles = {}

        def ab_r_parts(tau):
            """Create ab[tau] and accumulate all its x-window contributions.

            Window for step tau is x[tau-2 .. tau]; wr[d] maps x_{tau-2+d}.
            Called from step tau-1's tail (or pre-loop for tau=0), so every
            xt tile referenced is already resident. The LAST issued matmul
            carries stop only for tau==0 (no h contribution there).
            """
            abt = ab_pool.tile([128, 256], F32, tag="ab", name=f"ab{tau}")
            ab_tiles[tau] = abt
            first = True
            if ab_bias_nonzero:
                nc.tensor.matmul(abt, ones_row, bab_s, start=True, stop=False)
                first = False
            terms = [(d, tau - 2 + d) for d in range(3) if tau - 2 + d >= 0]
            for i, (d, tx) in enumerate(terms):
                last = i == len(terms) - 1
                nc.tensor.matmul(
                    abt, xt_t(tx), wr_s[:, d, :],
                    start=first, stop=(last and tau == 0),
                )
                first = False

        ab_r_parts(0)

        hT01 = None   # SBUF [128, 256] transposed h, scalar-copied (wab/y)
        hT01g = None  # SBUF [128, 256] transposed h, vector-copied (gates)
        g_lo = g_hi = None

        for t in range(T):
            if t % GB == 0 and (t // GB + 1) < NGROUPS:
                dma_group(t // GB + 1)

            # -- head: h-dependent matmuls (h = state entering step t) -------
            # wab/y read the scalar-copied hT01; the gates-h matmuls read the
            # vector-copied hT01g, which lands later, so the greedy scheduler
            # cannot slip a 512-wide gate matmul in front of the ab stop.
            g_lo = g_pool.tile([128, 512], F32, tag="glo")  # cols: f | i
            g_hi = g_pool.tile([128, 512], F32, tag="ghi")  # cols: o | g
            if t >= 1:
                hT0 = hT01[:, 0:128]
                hT1 = hT01[:, 128:256]
                nc.tensor.matmul(ab_tiles[t], hT0, wab_s[:, 0, :],
                                 start=False, stop=False)
                nc.tensor.matmul(y_ps, hT0, wp_s[:, 0, :],
                                 start=(t == 1), stop=False)
                nc.tensor.matmul(ab_tiles[t], hT1, wab_s[:, 1, :],
                                 start=False, stop=True)
                nc.tensor.matmul(y_ps, hT1, wp_s[:, 1, :],
                                 start=False, stop=False)
                g_started = False
                if gate_bias_nonzero:
                    nc.tensor.matmul(g_lo, ones_row, bg_s[:, 0:512],
                                     start=True, stop=False)
                    nc.tensor.matmul(g_hi, ones_row, bg_s[:, 512:1024],
                                     start=True, stop=False)
                    g_started = True
                hG0 = hT01g[:, 0:128]
                hG1 = hT01g[:, 128:256]
                nc.tensor.matmul(g_lo, hG0, wg_s[:, 1, 0:512],
                                 start=not g_started, stop=False)
                nc.tensor.matmul(g_hi, hG0, wg_s[:, 1, 512:1024],
                                 start=not g_started, stop=False)
                nc.tensor.matmul(g_lo, hG1, wg_s[:, 2, 0:512],
                                 start=False, stop=False)
                nc.tensor.matmul(g_hi, hG1, wg_s[:, 2, 512:1024],
                                 start=False, stop=False)
            elif gate_bias_nonzero:
                nc.tensor.matmul(g_lo, ones_row, bg_s[:, 0:512],
                                 start=True, stop=False)
                nc.tensor.matmul(g_hi, ones_row, bg_s[:, 512:1024],
                                 start=True, stop=False)

            # -- attention softmax path --------------------------------------
            abt = ab_tiles[t]
            t_ab = work.tile([128, 131], F32, tag="tab")
            nc.scalar.activation(out=t_ab, in_=abt[:, 0:131], func=AF.Tanh)
            e_a = work.tile([128, 3], F32, tag="ea")
            nc.scalar.activation(out=e_a, in_=t_ab[:, 0:3], func=AF.Exp)
            e_b = work.tile([128, 128], F32, tag="eb")
            s_b = work.tile([128, 1], F32, tag="sb")
            nc.scalar.activation(out=e_b, in_=t_ab[:, 3:131], func=AF.Exp,
                                 accum_out=s_b)

            # Vector chain. The r-branch (S_a adds -> mult -> recip) binds the
            # z time, so it is emitted ahead of everything it can precede:
            # adds first (tiny, unblock S_a), then U2, then mult/recip, then
            # the u MADD, then z.
            sa01 = work.tile([128, 1], F32, tag="sa01")
            nc.vector.tensor_add(sa01, e_a[:, 0:1], e_a[:, 1:2])
            s_a = work.tile([128, 1], F32, tag="sa")
            nc.vector.tensor_add(s_a, sa01, e_a[:, 2:3])
            u = work.tile([128, 128], F32, tag="u")
            u01 = None
            if t == 0:
                nc.vector.tensor_scalar_mul(u, xn_t(0), e_a[:, 2:3])
            elif t == 1:
                nc.vector._custom_dve(
                    U2_OP, out=u, in0=xn_t(0), in1=xn_t(1),
                    s0=e_a[:, 1:2], s1=e_a[:, 2:3])
            else:
                u01 = work.tile([128, 128], F32, tag="u01")
                nc.vector._custom_dve(
                    U2_OP, out=u01, in0=xn_t(t - 2), in1=xn_t(t - 1),
                    s0=e_a[:, 0:1], s1=e_a[:, 1:2])
            s_ab = work.tile([128, 1], F32, tag="sab")
            nc.vector.tensor_mul(s_ab, s_a, s_b)
            r_ab = work.tile([128, 1], F32, tag="rab")
            nc.vector.reciprocal(r_ab, s_ab)
            if u01 is not None:
                nc.vector.scalar_tensor_tensor(
                    out=u, in0=xn_t(t), scalar=e_a[:, 2:3], in1=u01,
                    op0=ALU.mult, op1=ALU.add)
            # z = e_beta * u * r  (normalized attention output)
            z = work.tile([128, 128], F32, tag="z")
            nc.vector.scalar_tensor_tensor(
                out=z, in0=u, scalar=r_ab, in1=e_b, op0=ALU.mult, op1=ALU.mult)

            # -- zT and gates-z ----------------------------------------------
            tr_z = tr_pool.tile([128, 128], F32, tag="trz")
            nc.tensor.transpose(tr_z, z, ident)
            zT = work.tile([128, 128], F32R, tag="zT")
            nc.vector.tensor_copy(out=zT, in_=tr_z)
            gz_start = t == 0 and not gate_bias_nonzero
            nc.tensor.matmul(g_lo, zT, wg_s[:, 0, 0:512],
                             start=gz_start, stop=True)
            nc.tensor.matmul(g_hi, zT, wg_s[:, 0, 512:1024],
                             start=gz_start, stop=True)

            # x-window contributions for the NEXT step's ab preacts ride the
            # PE while the gate activations below run on scalar/vector.
            if t + 1 < T:
                ab_r_parts(t + 1)

            # -- PE keep-warm fillers. Each burst is anchored on a tile this
            # step produces, so it releases only inside this step (a dep-free
            # filler would be drained by the scheduler at program start).
            # They keep the HAM activity window from seeing an idle PE, which
            # would throttle the PE clock to half rate.
            fill_ps = fill_pool.tile([128, 512], F32, tag="fill")
            # e_b-anchored: bridge the gap between the R matmuls and tr-z
            for _ in range(FILL_E):
                nc.tensor.matmul(fill_ps[:, 0:128], e_b, ident,
                                 start=True, stop=True)
            # zT-anchored (stationary already loaded for Gz): fill the
            # LSTM-elementwise tail
            for _ in range(FILL_Z):
                nc.tensor.matmul(fill_ps, zT, wg_s[:, 0, 0:512],
                                 start=True, stop=True)

            # -- gate activations + LSTM state update (doubled state) --------
            # g_lo cols: f 0:256, i 256:512; g_hi cols: o 0:256, g 256:512
            tg_fi = work.tile([128, 512], F32, tag="tgfi")
            nc.scalar.activation(out=tg_fi, in_=g_lo, func=AF.Tanh, scale=0.5)
            # A = (1+tanh(f/2)) * C   (= 4*sig(f)*c)
            A_t = work.tile([128, 256], F32, tag="A")
            nc.vector.scalar_tensor_tensor(
                out=A_t, in0=tg_fi[:, 0:256], scalar=1.0, in1=C_s,
                op0=ALU.add, op1=ALU.mult)
            tg_g = work.tile([128, 256], F32, tag="tgg")
            nc.scalar.activation(out=tg_g, in_=g_hi[:, 256:512], func=AF.Tanh)
            # Q = (1+tanh(i/2)) * tanh(g)   (= 2*sig(i)*tanh(g))
            Q_t = work.tile([128, 256], F32, tag="Q")
            nc.vector.scalar_tensor_tensor(
                out=Q_t, in0=tg_fi[:, 256:512], scalar=1.0, in1=tg_g,
                op0=ALU.add, op1=ALU.mult)
            tg_o = work.tile([128, 256], F32, tag="tgo")
            nc.scalar.activation(out=tg_o, in_=g_hi[:, 0:256], func=AF.Tanh,
                                 scale=0.5)
            # C_new = 0.5*A + Q   (= 2*c_new)
            nc.vector.scalar_tensor_tensor(
                out=C_s, in0=A_t, scalar=0.5, in1=Q_t,
                op0=ALU.mult, op1=ALU.add)
            # tanh(c_new) = tanh(0.5*C)
            t_c = work.tile([128, 256], F32, tag="tc")
            nc.scalar.activation(out=t_c, in_=C_s, func=AF.Tanh, scale=0.5)
            # H_new = (1+tanh(o/2)) * tanh(c_new)   (= 2*h_new)
            H_s = work.tile([128, 256], F32, tag="H")
            nc.vector.scalar_tensor_tensor(
                out=H_s, in0=tg_o, scalar=1.0, in1=t_c,
                op0=ALU.add, op1=ALU.mult)

            # -- transpose new h; scalar copies feed wab/y, vector copy ------
            # feeds the gates-h matmuls (see head comment).
            tr_h = tr_pool.tile([128, 256], F32, tag="trh")
            nc.tensor.transpose(tr_h[:, 0:128], H_s[:, 0:128], ident)
            nc.tensor.transpose(tr_h[:, 128:256], H_s[:, 128:256], ident)
            hT01 = work.tile([128, 256], F32R, tag="hT01")
            nc.scalar.copy(out=hT01[:, 0:128], in_=tr_h[:, 0:128])
            nc.scalar.copy(out=hT01[:, 128:256], in_=tr_h[:, 128:256])
            hT01g = work.tile([128, 256], F32R, tag="hT01g")
            nc.vector.tensor_copy(out=hT01g, in_=tr_h[:, 0:256])

            # fillers anchored on the new H: bridge the hT-copy latency window
            for _ in range(FILL_H):
                nc.tensor.transpose(fill_ps[:, 0:128], H_s[:, 0:128], ident)

        # ---- final h contribution to y + writeback --------------------------
        nc.tensor.matmul(y_ps, hT01[:, 0:128], wp_s[:, 0, :],
                         start=False, stop=False)
        nc.tensor.matmul(y_ps, hT01[:, 128:256], wp_s[:, 1, :],
                         start=False, stop=True)
        y_sb = work.tile([128, 1], F32, tag="ysb")
        nc.scalar.copy(out=y_sb, in_=y_ps[:, 0:1])
        nc.sync.dma_start(out=y_d[:], in_=y_sb)

    nc.finalize()
    return nc


def _prep_inputs(v, w_h_alpha, b_h_alpha, w_a_alpha, b_a_alpha,
                 w_h_beta, b_h_beta, w_a_beta, b_a_beta,
                 w_ih, b_ih, w_hh, b_hh, w_pred, b_pred):
    v = np.ascontiguousarray(np.asarray(v, dtype=np.float32))
    # gate row reorder: torch order (i,f,g,o) -> (f,i,o,g)
    idx = np.concatenate([np.arange(H, 2 * H), np.arange(0, H),
                          np.arange(3 * H, 4 * H), np.arange(2 * H, 3 * H)])
    wih_p = np.asarray(w_ih, np.float32)[idx]          # [1024, 128]
    whh_p = np.asarray(w_hh, np.float32)[idx]          # [1024, 256]
    bg = (np.asarray(b_ih, np.float32) + np.asarray(b_hh, np.float32))[idx]

    wg = np.zeros((3, 128, 1024), np.float32)
    wg[0] = wih_p.T
    wg[1] = 0.5 * whh_p.T[0:128]
    wg[2] = 0.5 * whh_p.T[128:256]

    wab = np.zeros((2, 128, 256), np.float32)
    wha_t = np.asarray(w_h_alpha, np.float32).T        # [H, 3]
    whb_t = np.asarray(w_h_beta, np.float32).T         # [H, F]
    for k in range(2):
        wab[k, :, 0:3] = 0.5 * wha_t[128 * k:128 * (k + 1)]
        wab[k, :, 3:131] = 0.5 * whb_t[128 * k:128 * (k + 1)]

    wr = np.zeros((3, 128, 256), np.float32)
    waa = np.asarray(w_a_alpha, np.float32)[0]         # [F]
    wab_beta = np.asarray(w_a_beta, np.float32)[0]     # [W]
    eye = np.eye(128, dtype=np.float32)
    for d in range(3):
        wr[d, :, d] = waa
        wr[d, :, 3:131] = wab_beta[d] * eye

    wp = np.zeros((2, 128, 8), np.float32)
    wp[:, :, 0] = (0.5 * np.asarray(w_pred, np.float32)[0]).reshape(2, 128)

    bab = np.zeros((1, 256), np.float32)
    bab[0, 0:3] = np.asarray(b_h_alpha, np.float32) + np.asarray(b_a_alpha,
                                                                 np.float32)[0]
    bab[0, 3:131] = np.asarray(b_h_beta, np.float32) + np.asarray(b_a_beta,
                                                                  np.float32)[0]

    gate_bias_nonzero = bool(np.any(bg != 0.0))
    ab_bias_nonzero = bool(np.any(bab != 0.0))

    shared = {
        "wg": wg, "wab": wab, "wr": wr, "wp": wp,
        "bg": bg.reshape(1, 1024), "bab": bab,
    }
    in_maps = []
    vs = v.reshape(NCORES, BL, T, F)
    for c in range(NCORES):
        vc = vs[c]                                     # [BL, T, F]
        in_maps.append({
            "xn": np.ascontiguousarray(vc.transpose(1, 0, 2)),  # [T, BL, F]
            "xt": np.ascontiguousarray(vc.transpose(1, 2, 0)),  # [T, F, BL]
            **shared,
        })
    b_pred_total = float(T) * np.asarray(b_pred, np.float32)    # [L]
    return in_maps, gate_bias_nonzero, ab_bias_nonzero, b_pred_total


def _run(inputs, trace=False):
    in_maps, gb_nz, ab_nz, b_pred_total = _prep_inputs(**inputs)
    key = (gb_nz, ab_nz)
    if key not in _CACHE:
        _CACHE[key] = build_kernel(gb_nz, ab_nz)
    nc = _CACHE[key]
    res = run_bass_kernel_spmd(
        nc, in_maps, core_ids=list(range(NCORES)), trace=trace,
    )
    y = np.concatenate([res.results[c]["y"] for c in range(NCORES)], axis=0)
    y = y + b_pred_total[None, :]
    return np.asarray(y, dtype=np.float32), res


def kernel(**inputs):
    y, _ = _run(inputs, trace=False)
    return y
